# revision 1
# baseline (speedup 1.0000x reference)
"""Masked-copy kernel for nn_CompactExpandModule on 8 Trainium2 NeuronCores.

out[b, s] = input_embeddings[b, s] if token_ids[b, s] in keep_token_ids else 0

keep_token_ids is a contiguous range (arange(16000) per the problem spec), so
membership is a single compare against a threshold, done on-device. Sharding is
pure data parallel: batch b -> core b (B == n_cores == 8).

Written in raw Bass (explicit semaphores): the walrus build in this container
encodes at most ONE sync wait per instruction, which rules out the Tile
framework's aggregated multi-wait drains. Raw `wait_ge` emits standalone
single-wait instructions, so every instruction stays within the limit.
"""

import sys

if "/opt/trn_rl_repo" not in sys.path:
    sys.path.insert(0, "/opt/trn_rl_repo")

import contextlib

import numpy as np

import concourse.bass as bass
import concourse.mybir as mybir
from concourse.bass_utils import run_bass_kernel_spmd

B, S, D = 8, 4096, 1024
P = 128            # SBUF partitions
C = 8              # seq rows per partition per tile
ROWS = P * C       # 512 rows per tile -> 2 MiB embedding tiles
NT = S // ROWS     # 8 tiles per core; all tiles single-use (16 MiB SBUF total)
N_CORES = 8

_program_cache: dict[tuple, bass.Bass] = {}


def _install_ntff_hook():
    """Register the axon NTFF profile hook that this image's boot skipped
    (its `antenv` package lacks `axon_hooks`). Mirrors trn_boot.py's
    `_ntff_profile_via_ctypes` against /opt/axon/libaxon_pjrt.so."""
    try:
        from antenv.axon_hooks import get_axon_ntff_profile_hook  # noqa: F401

        return True
    except ImportError:
        pass
    import ctypes
    import types

    try:
        lib = ctypes.CDLL("/opt/axon/libaxon_pjrt.so")
    except OSError:
        return False
    if not hasattr(lib, "axon_start_nrt_profile"):
        return False
    lib.axon_start_nrt_profile.argtypes = [
        ctypes.POINTER(ctypes.c_int64),
        ctypes.c_size_t,
    ]
    lib.axon_start_nrt_profile.restype = ctypes.c_int64
    lib.axon_stop_nrt_profile.argtypes = [ctypes.c_char_p]
    lib.axon_stop_nrt_profile.restype = ctypes.c_int64

    @contextlib.contextmanager
    def _hook(output_dir, device_ids):
        import jax

        jax.devices()
        if device_ids:
            ids = (ctypes.c_int64 * len(device_ids))(*device_ids)
            rc = lib.axon_start_nrt_profile(ids, len(device_ids))
        else:
            rc = lib.axon_start_nrt_profile(None, 0)
        if rc != 0:
            raise RuntimeError(f"axon_start_nrt_profile rc={rc}")
        try:
            yield
        finally:
            n = lib.axon_stop_nrt_profile(str(output_dir).encode())
            print(f"profile: {n} file(s) written to {output_dir}", file=sys.stderr)

    import antenv

    mod = types.ModuleType("antenv.axon_hooks")
    _state = {"hook": _hook}
    mod.set_axon_ntff_profile_hook = lambda h: _state.__setitem__("hook", h)
    mod.get_axon_ntff_profile_hook = lambda: _state["hook"]
    sys.modules["antenv.axon_hooks"] = mod
    antenv.axon_hooks = mod
    return True


def _build_program(lo: int, hi: int) -> bass.Bass:
    """One-core program: out = emb * (lo <= tok < hi), rows masked per token.

    Tile t covers rows [t*ROWS, (t+1)*ROWS); partition p holds rows
    t*ROWS + p*C .. +C-1 contiguously (16 KiB per partition per DMA).

    Pipeline: SP issues all loads up front (HWDGE); DVE computes each tile as
    its loads land; Pool (SWDGE) stores each tile as its compute finishes.
    """
    key = (lo, hi)
    if key in _program_cache:
        return _program_cache[key]

    nc = bass.Bass()
    emb = nc.declare_dram_parameter("emb", [S, D], mybir.dt.float32, isOutput=False)
    tok = nc.declare_dram_parameter("tok", [S], mybir.dt.int32, isOutput=False)
    out = nc.declare_dram_parameter("out", [S, D], mybir.dt.float32, isOutput=True)

    emb_t, out_t, tok_t = [], [], []
    for t in range(NT):
        r0 = t * ROWS
        emb_t.append(emb[r0 : r0 + ROWS, :].rearrange("(p c) d -> p c d", p=P))
        out_t.append(out[r0 : r0 + ROWS, :].rearrange("(p c) d -> p c d", p=P))
        tok_t.append(tok[r0 : r0 + ROWS].rearrange("(p c) -> p c", p=P))

    with contextlib.ExitStack() as ctx:
        data = [
            ctx.enter_context(
                nc.sbuf_tensor(f"data{t}", [P, C, D], mybir.dt.float32)
            )
            for t in range(NT)
        ]
        toks = [
            ctx.enter_context(nc.sbuf_tensor(f"tokt{t}", [P, C], mybir.dt.int32))
            for t in range(NT)
        ]
        masks = [
            ctx.enter_context(nc.sbuf_tensor(f"mask{t}", [P, C], mybir.dt.float32))
            for t in range(NT)
        ]
        # One semaphore per tile: both loads (+16 each) then the TT (+1).
        # A semaphore update may REACH a value another engine is waiting on,
        # but must never overshoot past a pending wait (CoreSim's semaphore
        # attribution rule) — per-tile sems make every wait an exact-reach.
        tsems = [
            ctx.enter_context(nc.semaphore(f"tile_sem{t}")) for t in range(NT)
        ]
        mask_sem = ctx.enter_context(nc.semaphore("mask_sem"))
        store_sem = ctx.enter_context(nc.semaphore("store_sem"))
        block = ctx.enter_context(nc.Block())

        @block.sync
        def _(sync: bass.BassEngine):
            for t in range(NT):
                sync.dma_start(out=data[t][:], in_=emb_t[t]).then_inc(tsems[t], 16)
                sync.dma_start(out=toks[t][:], in_=tok_t[t]).then_inc(tsems[t], 16)

        @block.vector
        def _(vector: bass.BassEngine):
            for t in range(NT):
                vector.wait_ge(tsems[t], 32)
                nc.vector.tensor_scalar(
                    out=masks[t][:], in0=toks[t][:], scalar1=hi, scalar2=None,
                    op0=mybir.AluOpType.is_lt,
                ).then_inc(mask_sem, 1)
                # DVE pipelines; a same-engine RAW (mask write -> read) still
                # needs a semaphore (CoreSim race detector flags it otherwise).
                vector.wait_ge(mask_sem, t + 1)
                nc.vector.tensor_tensor(
                    out=data[t][:], in0=data[t][:],
                    in1=masks[t][:].broadcast_to([P, C, D]),
                    op=mybir.AluOpType.mult,
                ).then_inc(tsems[t], 1)

        @block.gpsimd
        def _(gpsimd: bass.BassEngine):
            for t in range(NT):
                gpsimd.wait_ge(tsems[t], 33)
                gpsimd.dma_start(out=out_t[t], in_=data[t][:]).then_inc(store_sem, 16)
            gpsimd.wait_ge(store_sem, 16 * NT)

    _program_cache[key] = nc
    return nc


def _keep_range(keep_token_ids: np.ndarray) -> tuple[int, int] | None:
    """If keep_token_ids is a contiguous integer range, return (lo, hi)."""
    k = np.asarray(keep_token_ids)
    if k.ndim != 1 or k.size == 0:
        return None
    lo = int(k.min())
    hi = int(k.max()) + 1
    if hi - lo == k.size and np.unique(k).size == k.size:
        return lo, hi
    return None


def kernel(input_embeddings, token_ids, keep_token_ids, _want_timing=False):
    emb = np.ascontiguousarray(np.asarray(input_embeddings, dtype=np.float32))
    tok = np.ascontiguousarray(np.asarray(token_ids, dtype=np.int32))
    keep = np.asarray(keep_token_ids)
    assert emb.shape == (B, S, D) and tok.shape == (B, S)

    rng = _keep_range(keep)
    if rng is None or rng[0] != 0:
        # Keep-set is not arange(0, k) (not expected per spec): remap token
        # ids on the host so the device threshold compare still yields isin().
        tok = np.where(np.isin(tok, keep), np.int32(0), np.int32(1)).astype(np.int32)
        lo, hi = 0, 1
    else:
        lo, hi = rng

    if _want_timing:
        _want_timing = _install_ntff_hook()
    nc = _build_program(lo, hi)
    in_maps = [{"emb": emb[b], "tok": tok[b]} for b in range(B)]
    res = run_bass_kernel_spmd(
        nc, in_maps, list(range(N_CORES)), trace=bool(_want_timing)
    )
    out = np.stack([np.asarray(res.results[b]["out"]) for b in range(B)], axis=0)
    if _want_timing:
        return out, res.exec_time_ns
    return out

